# revision 6
# baseline (speedup 1.0000x reference)
"""Trainium2 Bass kernel for DirectionalConvLayer.

Problem: 4 directional 3-tap convs over [256, 256, 15, 15] images, one input
per direction (horizontal / vertical / main-diagonal / anti-diagonal taps),
shared weight [256, 256, 3] and bias [256].

Strategy: every direction is a 1-D 3-tap conv along its set of lines
(rows / columns / diagonals / anti-diagonals) with a dense 256x256 channel
mix per tap. On the host, each image's lines are packed into a flat 256-col
buffer (leading zero + lines separated by single zeros), which makes the
device kernel direction-agnostic: a pure 3-tap conv along the flat axis,
i.e. accumulating matmuls against +/-1-shifted views. One SPMD program on
8 cores: core = direction * 2 + batch-half (128 images each).

Device loop per core: 32 chunks of 4 images (1024 cols). Per chunk: 1 DMA in,
2 free-tiles x 2 cout-chunks x (3 taps x 2 cin-chunks) accumulating matmuls
into PSUM (float32r operands -> full-rate fp32), bias-add PSUM->SBUF on the
vector engine, 1 DMA out.
"""
import os
from contextlib import ExitStack

import numpy as np

import concourse.bass as bass
import concourse.tile as tile
from concourse import mybir
from concourse.bass_utils import run_bass_kernel_spmd

P = 128
NIMG = 128            # images per core
IMGC = 256            # packed cols per image
NCOL = NIMG * IMGC    # 32768
CH_IMG = 4
CH_COL = CH_IMG * IMGC  # 1024
NCH = NIMG // CH_IMG    # 32
FT = 512
NFT = CH_COL // FT      # 2

H = W = 15

# dtype used for the matmul operands (fp32 bits reinterpreted):
# float32r streams at 1 row/cycle for free-dim >= 256 (plain float32 is 4x).
MM_DT = mybir.dt.float32r


def _build_maps():
    maps = []
    for d in range(4):
        if d == 0:
            lines = [[(i, j) for j in range(15)] for i in range(15)]
        elif d == 1:
            lines = [[(i, j) for i in range(15)] for j in range(15)]
        elif d == 2:
            lines = [
                [(i, i - k) for i in range(max(0, k), min(15, 15 + k))]
                for k in range(-14, 15)
            ]
        else:
            lines = [
                [(i, s - i) for i in range(max(0, s - 14), min(15, s + 1))]
                for s in range(29)
            ]
        pos2flat = np.full(IMGC, -1, np.int64)
        p = 1
        for ln in lines:
            for (i, j) in ln:
                pos2flat[p] = i * W + j
                p += 1
            p += 1  # separator zero
        assert p <= IMGC + 1
        flat2pos = np.zeros(H * W, np.int64)
        for pos, f in enumerate(pos2flat):
            if f >= 0:
                flat2pos[f] = pos
        maps.append((pos2flat, flat2pos))
    return maps


_MAPS = _build_maps()


def _split_drain_waits(nc, max_waits=1):
    """Workaround for this walrus build's 'Too many sync wait commands' limit
    (1 sync wait per instruction): hoist excess sem-waits onto nop
    instructions inserted right before the instruction on the same engine.
    Sequential waits on one engine queue are equivalent to multiple waits on
    one instruction."""
    n = 0
    for fn in nc.m.functions:
        for bb in fn.blocks:
            insts = bb.instructions
            i = 0
            while i < len(insts):
                inst = insts[i]
                si = inst.sync_info
                if si is not None and si.on_wait and len(si.on_wait) > max_waits:
                    extra = list(si.on_wait)[max_waits:]
                    si.on_wait = list(si.on_wait)[:max_waits]
                    for wt in extra:
                        nop = mybir.InstNoOp(
                            name=f"I-waitsplit-{n}",
                            engine=inst.engine,
                            sync_info=mybir.SyncInfo(on_wait=[wt], on_update=[]),
                        )
                        nc.register_instruction(nop)
                        n += 1
                        insts.insert(i, nop)
                        i += 1
                i += 1
    return n


def build_program():
    nc = bass.Bass("TRN2", target_bir_lowering=False, debug=False, num_devices=8)
    xin = nc.dram_tensor(
        "xin", [P, 2, NCOL + 2], MM_DT, kind="ExternalInput"
    ).ap()
    wts = nc.dram_tensor(
        "wts", [P, 3, 2, 256], MM_DT, kind="ExternalInput"
    ).ap()
    bin_ = nc.dram_tensor("bin", [P, 2], mybir.dt.float32, kind="ExternalInput").ap()
    yout = nc.dram_tensor(
        "yout", [P, 2, NCOL], mybir.dt.float32, kind="ExternalOutput"
    ).ap()

    with tile.TileContext(nc) as tc, ExitStack() as ctx:
        cpool = ctx.enter_context(tc.tile_pool(name="const", bufs=1))
        xpool = ctx.enter_context(tc.tile_pool(name="x", bufs=3))
        ypool = ctx.enter_context(tc.tile_pool(name="y", bufs=3))
        ppool = ctx.enter_context(tc.tile_pool(name="ps", bufs=8, space="PSUM"))

        wt = cpool.tile([P, 3, 2, 256], MM_DT)
        nc.sync.dma_start(wt[:], wts[:])
        bt = cpool.tile([P, 2], mybir.dt.float32)
        nc.sync.dma_start(bt[:], bin_[:])

        for c in range(NCH):
            xt = xpool.tile([P, 2, CH_COL + 2], MM_DT)
            nc.sync.dma_start(
                xt[:], xin[:, :, c * CH_COL : c * CH_COL + CH_COL + 2]
            )
            yt = ypool.tile([P, 2, CH_COL], mybir.dt.float32)
            for f in range(NFT):
                for o in range(2):
                    ps = ppool.tile([P, FT], mybir.dt.float32)
                    g = 0
                    for t in range(3):
                        for k in range(2):
                            lhsT = wt[:, t, k, o * P : (o + 1) * P]
                            base = 1 + f * FT + (t - 1)
                            rhs = xt[:, k, base : base + FT]
                            nc.tensor.matmul(
                                ps[:], lhsT, rhs, start=(g == 0), stop=(g == 5)
                            )
                            g += 1
                    nc.vector.tensor_scalar_add(
                        yt[:, o, f * FT : (f + 1) * FT], ps[:], bt[:, o : o + 1]
                    )
            nc.sync.dma_start(yout[:, :, c * CH_COL : (c + 1) * CH_COL], yt[:])
    _split_drain_waits(nc)
    return nc


def pack_inputs(xs, weight, bias):
    """xs: list of 4 arrays [256, 256, 15, 15]. Returns in_maps for cores 0-7."""
    # w_dev[p, t, k, o] = weight[o, k*128+p, t]
    w_dev = np.ascontiguousarray(
        weight.transpose(1, 2, 0).reshape(2, P, 3, 256).transpose(1, 2, 0, 3)
    )
    b_dev = np.ascontiguousarray(bias.reshape(2, P).T)
    in_maps = []
    for d in range(4):
        pos2flat = _MAPS[d][0]
        idx = np.where(pos2flat >= 0, pos2flat, H * W)
        x = xs[d]
        B, C = x.shape[:2]
        xf = x.reshape(B, C, H * W)
        xaug = np.concatenate([xf, np.zeros((B, C, 1), np.float32)], axis=2)
        packed = xaug[:, :, idx]  # [B, C, IMGC]
        for h in range(2):
            sh = packed[h * NIMG : (h + 1) * NIMG]          # [128, C, IMGC]
            arr = sh.transpose(1, 0, 2).reshape(C, NCOL)     # [C, NCOL]
            xin_np = np.zeros((P, 2, NCOL + 2), np.float32)
            xin_np[:, 0, 1 : NCOL + 1] = arr[:P]
            xin_np[:, 1, 1 : NCOL + 1] = arr[P:]
            in_maps.append({"xin": xin_np, "wts": w_dev, "bin": b_dev})
    return in_maps


def unpack_outputs(results):
    outs = []
    for d in range(4):
        flat2pos = _MAPS[d][1]
        y = np.empty((256, 256, H, W), np.float32)
        for h in range(2):
            yo = results[d * 2 + h]["yout"]                   # [128, 2, NCOL]
            yc = np.asarray(yo).transpose(1, 0, 2).reshape(256, NIMG, IMGC)
            take = yc[:, :, flat2pos]                         # [256, 128, 225]
            y[h * NIMG : (h + 1) * NIMG] = take.transpose(1, 0, 2).reshape(
                NIMG, 256, H, W
            )
        outs.append(y)
    return tuple(outs)


def kernel(x0, x1, x2, x3, weight, bias):
    xs = [np.ascontiguousarray(np.asarray(a, dtype=np.float32)) for a in (x0, x1, x2, x3)]
    weight = np.asarray(weight, dtype=np.float32)
    bias = np.asarray(bias, dtype=np.float32)

    nc = build_program()
    in_maps = pack_inputs(xs, weight, bias)
    res = run_bass_kernel_spmd(nc, in_maps, list(range(8)))
    return unpack_outputs(res.results)


# revision 8
# speedup vs baseline: 1.1534x; 1.1534x over previous
"""Trainium2 Bass kernel for DirectionalConvLayer.

Problem: 4 directional 3-tap convs over [256, 256, 15, 15] images, one input
per direction (horizontal / vertical / main-diagonal / anti-diagonal taps),
shared weight [256, 256, 3] and bias [256].

Strategy: every direction is a 1-D 3-tap conv along its set of lines
(rows / columns / diagonals / anti-diagonals) with a dense 256x256 channel
mix per tap. On the host, each image's lines are packed into a flat 256-col
buffer (leading zero + lines separated by single zeros), which makes the
device kernel direction-agnostic: a pure 3-tap conv along the flat axis,
i.e. accumulating matmuls against +/-1-shifted views. One SPMD program on
8 cores: core = direction * 2 + batch-half (128 images each).

Device loop per core: 32 chunks of 4 images (1024 cols). Per chunk: 1 DMA in,
2 free-tiles x 2 cout-chunks x (3 taps x 2 cin-chunks) accumulating matmuls
into PSUM (float32r operands -> full-rate fp32), bias-add PSUM->SBUF on the
vector engine, 1 DMA out.
"""
import os
from contextlib import ExitStack

import numpy as np

import concourse.bass as bass
import concourse.tile as tile
from concourse import mybir
from concourse.bass_utils import run_bass_kernel_spmd

P = 128
NIMG = 128            # images per core
IMGC = 256            # packed cols per image
NCOL = NIMG * IMGC    # 32768
CH_IMG = 4
CH_COL = CH_IMG * IMGC  # 1024
NCH = NIMG // CH_IMG    # 32
FT = 512
NFT = CH_COL // FT      # 2

H = W = 15

# dtype used for the matmul operands (fp32 bits reinterpreted):
# float32r streams at 1 row/cycle for free-dim >= 256 (plain float32 is 4x).
MM_DT = mybir.dt.float32r


def _build_maps():
    maps = []
    for d in range(4):
        if d == 0:
            lines = [[(i, j) for j in range(15)] for i in range(15)]
        elif d == 1:
            lines = [[(i, j) for i in range(15)] for j in range(15)]
        elif d == 2:
            lines = [
                [(i, i - k) for i in range(max(0, k), min(15, 15 + k))]
                for k in range(-14, 15)
            ]
        else:
            lines = [
                [(i, s - i) for i in range(max(0, s - 14), min(15, s + 1))]
                for s in range(29)
            ]
        pos2flat = np.full(IMGC, -1, np.int64)
        p = 1
        for ln in lines:
            for (i, j) in ln:
                pos2flat[p] = i * W + j
                p += 1
            p += 1  # separator zero
        assert p <= IMGC + 1
        flat2pos = np.zeros(H * W, np.int64)
        for pos, f in enumerate(pos2flat):
            if f >= 0:
                flat2pos[f] = pos
        maps.append((pos2flat, flat2pos))
    return maps


_MAPS = _build_maps()


def _split_drain_waits(nc, max_waits=1):
    """Workaround for this walrus build's 'Too many sync wait commands' limit
    (1 sync wait per instruction): hoist excess sem-waits onto nop
    instructions inserted right before the instruction on the same engine.
    Sequential waits on one engine queue are equivalent to multiple waits on
    one instruction."""
    n = 0
    for fn in nc.m.functions:
        for bb in fn.blocks:
            insts = bb.instructions
            i = 0
            while i < len(insts):
                inst = insts[i]
                si = inst.sync_info
                if si is not None and si.on_wait and len(si.on_wait) > max_waits:
                    extra = list(si.on_wait)[max_waits:]
                    si.on_wait = list(si.on_wait)[:max_waits]
                    for wt in extra:
                        nop = mybir.InstNoOp(
                            name=f"I-waitsplit-{n}",
                            engine=inst.engine,
                            sync_info=mybir.SyncInfo(on_wait=[wt], on_update=[]),
                        )
                        nc.register_instruction(nop)
                        n += 1
                        insts.insert(i, nop)
                        i += 1
                i += 1
    return n


def build_program():
    nc = bass.Bass("TRN2", target_bir_lowering=False, debug=False, num_devices=8)
    xin = nc.dram_tensor(
        "xin", [P, 2, NCOL + 2], MM_DT, kind="ExternalInput"
    ).ap()
    wts = nc.dram_tensor(
        "wts", [P, 3, 2, 256], MM_DT, kind="ExternalInput"
    ).ap()
    yout = nc.dram_tensor(
        "yout", [P, 2, NCOL], mybir.dt.float32, kind="ExternalOutput"
    ).ap()

    # DMA unit sizes in images: small prologue units to prime the pipeline
    # quickly, large middle units for DMA efficiency, small epilogue units to
    # shorten the tail drain. Must sum to NIMG and each must be even (one
    # free-tile = 2 images).
    units = [2, 2, 4] + [8] * 14 + [4, 2, 2]
    assert sum(units) == NIMG

    with tile.TileContext(nc) as tc, ExitStack() as ctx:
        cpool = ctx.enter_context(tc.tile_pool(name="const", bufs=1))
        xpool = ctx.enter_context(tc.tile_pool(name="x", bufs=3))
        ypool = ctx.enter_context(tc.tile_pool(name="y", bufs=3))
        ppool = ctx.enter_context(tc.tile_pool(name="ps", bufs=8, space="PSUM"))

        wt = cpool.tile([P, 3, 2, 256], MM_DT)
        nc.sync.dma_start(wt[:], wts[:])

        img0 = 0
        for u in units:
            ucol = u * IMGC
            base_col = img0 * IMGC
            xt = xpool.tile([P, 2, ucol + 2], MM_DT)
            nc.sync.dma_start(xt[:], xin[:, :, base_col : base_col + ucol + 2])
            yt = ypool.tile([P, 2, ucol], mybir.dt.float32)
            for f in range(ucol // FT):
                for o in range(2):
                    ps = ppool.tile([P, FT], mybir.dt.float32)
                    g = 0
                    for t in range(3):
                        for k in range(2):
                            lhsT = wt[:, t, k, o * P : (o + 1) * P]
                            rhs = xt[:, k, f * FT + t : f * FT + t + FT]
                            nc.tensor.matmul(
                                ps[:], lhsT, rhs, start=(g == 0), stop=(g == 5)
                            )
                            g += 1
                    nc.vector.tensor_copy(yt[:, o, f * FT : (f + 1) * FT], ps[:])
            nc.sync.dma_start(yout[:, :, base_col : base_col + ucol], yt[:])
            img0 += u
    _split_drain_waits(nc)
    return nc


def pack_inputs(xs, weight):
    """xs: list of 4 arrays [256, 256, 15, 15]. Returns in_maps for cores 0-7."""
    # w_dev[p, t, k, o] = weight[o, k*128+p, t]
    w_dev = np.ascontiguousarray(
        weight.transpose(1, 2, 0).reshape(2, P, 3, 256).transpose(1, 2, 0, 3)
    )
    in_maps = []
    for d in range(4):
        pos2flat = _MAPS[d][0]
        idx = np.where(pos2flat >= 0, pos2flat, H * W)
        x = xs[d]
        B, C = x.shape[:2]
        xf = x.reshape(B, C, H * W)
        xaug = np.concatenate([xf, np.zeros((B, C, 1), np.float32)], axis=2)
        packed = xaug[:, :, idx]  # [B, C, IMGC]
        for h in range(2):
            sh = packed[h * NIMG : (h + 1) * NIMG]          # [128, C, IMGC]
            arr = sh.transpose(1, 0, 2).reshape(C, NCOL)     # [C, NCOL]
            xin_np = np.zeros((P, 2, NCOL + 2), np.float32)
            xin_np[:, 0, 1 : NCOL + 1] = arr[:P]
            xin_np[:, 1, 1 : NCOL + 1] = arr[P:]
            in_maps.append({"xin": xin_np, "wts": w_dev})
    return in_maps


def unpack_outputs(results, bias):
    outs = []
    b = bias[None, :, None].astype(np.float32)                # [1, 256, 1]
    for d in range(4):
        flat2pos = _MAPS[d][1]
        y = np.empty((256, 256, H, W), np.float32)
        for h in range(2):
            yo = results[d * 2 + h]["yout"]                   # [128, 2, NCOL]
            yc = np.asarray(yo).transpose(1, 0, 2).reshape(256, NIMG, IMGC)
            take = yc[:, :, flat2pos]                         # [256, 128, 225]
            take = take.transpose(1, 0, 2) + b                # [128, 256, 225]
            y[h * NIMG : (h + 1) * NIMG] = take.reshape(NIMG, 256, H, W)
        outs.append(y)
    return tuple(outs)


def kernel(x0, x1, x2, x3, weight, bias):
    xs = [np.ascontiguousarray(np.asarray(a, dtype=np.float32)) for a in (x0, x1, x2, x3)]
    weight = np.asarray(weight, dtype=np.float32)
    bias = np.asarray(bias, dtype=np.float32)

    nc = build_program()
    in_maps = pack_inputs(xs, weight)
    res = run_bass_kernel_spmd(nc, in_maps, list(range(8)))
    return unpack_outputs(res.results, bias)


# revision 9
# speedup vs baseline: 1.3927x; 1.2074x over previous
"""Trainium2 Bass kernel for DirectionalConvLayer.

Problem: 4 directional 3-tap convs over [256, 256, 15, 15] images, one input
per direction (horizontal / vertical / main-diagonal / anti-diagonal taps),
shared weight [256, 256, 3] and bias [256].

Strategy: every direction is a 1-D 3-tap conv along its set of lines
(rows / columns / diagonals / anti-diagonals) with a dense 256x256 channel
mix per tap. On the host, each image's lines are packed into a flat 256-col
buffer (leading zero + lines separated by single zeros), which makes the
device kernel direction-agnostic: a pure 3-tap conv along the flat axis,
i.e. accumulating matmuls against +/-1-shifted views. One SPMD program on
8 cores: core = direction * 2 + batch-half (128 images each).

Device loop per core: 32 chunks of 4 images (1024 cols). Per chunk: 1 DMA in,
2 free-tiles x 2 cout-chunks x (3 taps x 2 cin-chunks) accumulating matmuls
into PSUM (float32r operands -> full-rate fp32), bias-add PSUM->SBUF on the
vector engine, 1 DMA out.
"""
import os
from contextlib import ExitStack

import numpy as np

import concourse.bass as bass
import concourse.tile as tile
from concourse import mybir
from concourse.bass_utils import run_bass_kernel_spmd

P = 128
NIMG = 128            # images per core
IMGC = 256            # packed cols per image
NCOL = NIMG * IMGC    # 32768
CH_IMG = 4
CH_COL = CH_IMG * IMGC  # 1024
NCH = NIMG // CH_IMG    # 32
FT = 512
NFT = CH_COL // FT      # 2

H = W = 15

# Matmul/transfer dtype. float16 keeps 10 explicit mantissa bits (within 2x
# of float32r's effective 11) at half the DMA bytes; the PE runs fp16 at the
# same 1 cycle/row as bf16/fp32r. PSUM accumulation stays fp32.
MM_DT = mybir.dt.float16
MM_NP = np.float16


def _build_maps():
    maps = []
    for d in range(4):
        if d == 0:
            lines = [[(i, j) for j in range(15)] for i in range(15)]
        elif d == 1:
            lines = [[(i, j) for i in range(15)] for j in range(15)]
        elif d == 2:
            lines = [
                [(i, i - k) for i in range(max(0, k), min(15, 15 + k))]
                for k in range(-14, 15)
            ]
        else:
            lines = [
                [(i, s - i) for i in range(max(0, s - 14), min(15, s + 1))]
                for s in range(29)
            ]
        pos2flat = np.full(IMGC, -1, np.int64)
        p = 1
        for ln in lines:
            for (i, j) in ln:
                pos2flat[p] = i * W + j
                p += 1
            p += 1  # separator zero
        assert p <= IMGC + 1
        flat2pos = np.zeros(H * W, np.int64)
        for pos, f in enumerate(pos2flat):
            if f >= 0:
                flat2pos[f] = pos
        maps.append((pos2flat, flat2pos))
    return maps


_MAPS = _build_maps()


def _split_drain_waits(nc, max_waits=1):
    """Workaround for this walrus build's 'Too many sync wait commands' limit
    (1 sync wait per instruction): hoist excess sem-waits onto nop
    instructions inserted right before the instruction on the same engine.
    Sequential waits on one engine queue are equivalent to multiple waits on
    one instruction."""
    n = 0
    for fn in nc.m.functions:
        for bb in fn.blocks:
            insts = bb.instructions
            i = 0
            while i < len(insts):
                inst = insts[i]
                si = inst.sync_info
                if si is not None and si.on_wait and len(si.on_wait) > max_waits:
                    extra = list(si.on_wait)[max_waits:]
                    si.on_wait = list(si.on_wait)[:max_waits]
                    for wt in extra:
                        nop = mybir.InstNoOp(
                            name=f"I-waitsplit-{n}",
                            engine=inst.engine,
                            sync_info=mybir.SyncInfo(on_wait=[wt], on_update=[]),
                        )
                        nc.register_instruction(nop)
                        n += 1
                        insts.insert(i, nop)
                        i += 1
                i += 1
    return n


def build_program():
    nc = bass.Bass("TRN2", target_bir_lowering=False, debug=False, num_devices=8)
    xin = nc.dram_tensor(
        "xin", [P, 2, NCOL + 2], MM_DT, kind="ExternalInput"
    ).ap()
    wts = nc.dram_tensor(
        "wts", [P, 3, 2, 256], MM_DT, kind="ExternalInput"
    ).ap()
    yout = nc.dram_tensor(
        "yout", [P, 2, NCOL], MM_DT, kind="ExternalOutput"
    ).ap()

    # DMA unit sizes in images: small prologue units to prime the pipeline
    # quickly, large middle units for DMA efficiency, small epilogue units to
    # shorten the tail drain. Must sum to NIMG and each must be even (one
    # free-tile = 2 images).
    units = [2, 2, 4] + [8] * 14 + [4, 2, 2]
    assert sum(units) == NIMG

    with tile.TileContext(nc) as tc, ExitStack() as ctx:
        cpool = ctx.enter_context(tc.tile_pool(name="const", bufs=1))
        xpool = ctx.enter_context(tc.tile_pool(name="x", bufs=3))
        ypool = ctx.enter_context(tc.tile_pool(name="y", bufs=3))
        ppool = ctx.enter_context(tc.tile_pool(name="ps", bufs=8, space="PSUM"))

        wt = cpool.tile([P, 3, 2, 256], MM_DT)
        nc.gpsimd.dma_start(wt[:], wts[:])

        img0 = 0
        for u in units:
            ucol = u * IMGC
            base_col = img0 * IMGC
            xt = xpool.tile([P, 2, ucol + 2], MM_DT)
            nc.sync.dma_start(xt[:], xin[:, :, base_col : base_col + ucol + 2])
            yt = ypool.tile([P, 2, ucol], MM_DT)
            for f in range(ucol // FT):
                for o in range(2):
                    ps = ppool.tile([P, FT], mybir.dt.float32)
                    g = 0
                    for t in range(3):
                        for k in range(2):
                            lhsT = wt[:, t, k, o * P : (o + 1) * P]
                            rhs = xt[:, k, f * FT + t : f * FT + t + FT]
                            nc.tensor.matmul(
                                ps[:], lhsT, rhs, start=(g == 0), stop=(g == 5)
                            )
                            g += 1
                    nc.vector.tensor_copy(yt[:, o, f * FT : (f + 1) * FT], ps[:])
            nc.sync.dma_start(yout[:, :, base_col : base_col + ucol], yt[:])
            img0 += u
    _split_drain_waits(nc)
    return nc


def pack_inputs(xs, weight):
    """xs: list of 4 arrays [256, 256, 15, 15]. Returns in_maps for cores 0-7."""
    # w_dev[p, t, k, o] = weight[o, k*128+p, t]
    w_dev = np.ascontiguousarray(
        weight.transpose(1, 2, 0).reshape(2, P, 3, 256).transpose(1, 2, 0, 3)
    )
    in_maps = []
    for d in range(4):
        pos2flat = _MAPS[d][0]
        idx = np.where(pos2flat >= 0, pos2flat, H * W)
        x = xs[d]
        B, C = x.shape[:2]
        xf = x.reshape(B, C, H * W)
        xaug = np.concatenate([xf, np.zeros((B, C, 1), np.float32)], axis=2)
        packed = xaug[:, :, idx]  # [B, C, IMGC]
        for h in range(2):
            sh = packed[h * NIMG : (h + 1) * NIMG]          # [128, C, IMGC]
            arr = sh.transpose(1, 0, 2).reshape(C, NCOL)     # [C, NCOL]
            xin_np = np.zeros((P, 2, NCOL + 2), MM_NP)
            xin_np[:, 0, 1 : NCOL + 1] = arr[:P]
            xin_np[:, 1, 1 : NCOL + 1] = arr[P:]
            in_maps.append({"xin": xin_np, "wts": w_dev.astype(MM_NP)})
    return in_maps


def unpack_outputs(results, bias):
    outs = []
    b = bias[None, :, None].astype(np.float32)                # [1, 256, 1]
    for d in range(4):
        flat2pos = _MAPS[d][1]
        y = np.empty((256, 256, H, W), np.float32)
        for h in range(2):
            yo = results[d * 2 + h]["yout"]                   # [128, 2, NCOL]
            yc = (
                np.asarray(yo)
                .astype(np.float32)
                .transpose(1, 0, 2)
                .reshape(256, NIMG, IMGC)
            )
            take = yc[:, :, flat2pos]                         # [256, 128, 225]
            take = take.transpose(1, 0, 2) + b                # [128, 256, 225]
            y[h * NIMG : (h + 1) * NIMG] = take.reshape(NIMG, 256, H, W)
        outs.append(y)
    return tuple(outs)


def kernel(x0, x1, x2, x3, weight, bias):
    xs = [np.ascontiguousarray(np.asarray(a, dtype=np.float32)) for a in (x0, x1, x2, x3)]
    weight = np.asarray(weight, dtype=np.float32)
    bias = np.asarray(bias, dtype=np.float32)

    nc = build_program()
    in_maps = pack_inputs(xs, weight)
    res = run_bass_kernel_spmd(nc, in_maps, list(range(8)))
    return unpack_outputs(res.results, bias)


# revision 10
# speedup vs baseline: 1.4303x; 1.0270x over previous
"""Trainium2 Bass kernel for DirectionalConvLayer.

Problem: 4 directional 3-tap convs over [256, 256, 15, 15] fp32 images, one
input per direction (horizontal / vertical / main-diagonal / anti-diagonal
taps), shared weight [256, 256, 3] and bias [256].

Strategy: every direction is a 1-D 3-tap conv along its set of lines
(rows / columns / diagonals / anti-diagonals) with a dense 256x256 channel
mix per tap. On the host, ALL lines of ALL four inputs are packed into one
flat stream (single zero separator between consecutive lines), split evenly
across 8 cores at line boundaries. The device kernel is direction-agnostic:
a pure 3-tap conv along the flat axis — accumulating matmuls against
+/-1-shifted views of the stream, contraction over C_in in two 128-chunks.

Transfers and matmul operands are float16 (10-bit mantissa, ~ the tensor
engine's own fp32r precision) at full 1-cycle/row PE rate; PSUM accumulates
in fp32. Per core: 62 free-tiles of 512 cols; per tile x 2 cout-chunks:
6 accumulating matmuls (3 taps x 2 cin-chunks), then a vector-engine
PSUM->SBUF cast and DMA out. Bias is added on the host during unpacking.
"""
from contextlib import ExitStack

import numpy as np

import concourse.bass as bass
import concourse.tile as tile
from concourse import mybir
from concourse.bass_utils import run_bass_kernel_spmd

P = 128
FT = 512
NTILE = 62
CORE_COLS = NTILE * FT     # 31744 cols per core
H = W = 15
NCORE = 8

MM_DT = mybir.dt.float16
MM_NP = np.float16


def _build_lines(d):
    if d == 0:
        return [[(i, j) for j in range(W)] for i in range(H)]
    if d == 1:
        return [[(i, j) for i in range(H)] for j in range(W)]
    if d == 2:
        return [
            [(i, i - k) for i in range(max(0, k), min(H, H + k))]
            for k in range(-(W - 1), W)
        ]
    return [
        [(i, s - i) for i in range(max(0, s - (W - 1)), min(H, s + 1))]
        for s in range(H + W - 1)
    ]


def _build_stream_map():
    """Greedy-pack every (direction, image, line) into NCORE x CORE_COLS.
    colmap[d, b, i*W+j] = core * CORE_COLS + local_col. Consecutive cells of
    a line are adjacent; one zero separator between lines; core slices start
    at line starts, so the +/-1 conv taps only ever cross into zeros."""
    colmap = np.full((4, 256, H * W), -1, np.int64)
    core, col = 0, 0
    for d in range(4):
        lines = _build_lines(d)
        for b in range(256):
            for ln in lines:
                ll = len(ln)
                if col + ll > CORE_COLS:
                    core += 1
                    col = 0
                    assert core < NCORE, "stream overflow"
                for i, (r, c) in enumerate(ln):
                    colmap[d, b, r * W + c] = core * CORE_COLS + col + i
                col += ll + 1
    assert (colmap >= 0).all()
    return colmap


_COLMAP = _build_stream_map()


def _split_drain_waits(nc, max_waits=1):
    """Workaround for this walrus build's 'Too many sync wait commands' limit
    (1 sync wait per instruction): hoist excess sem-waits onto nop
    instructions inserted right before the instruction on the same engine.
    Sequential waits on one engine queue are equivalent to multiple waits on
    one instruction."""
    n = 0
    for fn in nc.m.functions:
        for bb in fn.blocks:
            insts = bb.instructions
            i = 0
            while i < len(insts):
                inst = insts[i]
                si = inst.sync_info
                if si is not None and si.on_wait and len(si.on_wait) > max_waits:
                    extra = list(si.on_wait)[max_waits:]
                    si.on_wait = list(si.on_wait)[:max_waits]
                    for wt in extra:
                        nop = mybir.InstNoOp(
                            name=f"I-waitsplit-{n}",
                            engine=inst.engine,
                            sync_info=mybir.SyncInfo(on_wait=[wt], on_update=[]),
                        )
                        nc.register_instruction(nop)
                        n += 1
                        insts.insert(i, nop)
                        i += 1
                i += 1
    return n


def build_program():
    nc = bass.Bass("TRN2", target_bir_lowering=False, debug=False, num_devices=8)
    xin = nc.dram_tensor(
        "xin", [P, 2, CORE_COLS + 2], MM_DT, kind="ExternalInput"
    ).ap()
    wts = nc.dram_tensor(
        "wts", [P, 3, 2, 256], MM_DT, kind="ExternalInput"
    ).ap()
    yout = nc.dram_tensor(
        "yout", [P, 2, CORE_COLS], MM_DT, kind="ExternalOutput"
    ).ap()

    # DMA unit sizes in free-tiles: small prologue units so the PE starts
    # quickly, large middle units for DMA efficiency, small epilogue units to
    # shorten the tail.
    units = [1, 1, 2] + [4] * 13 + [2, 2, 1, 1]
    assert sum(units) == NTILE

    with tile.TileContext(nc) as tc, ExitStack() as ctx:
        cpool = ctx.enter_context(tc.tile_pool(name="const", bufs=1))
        xpool = ctx.enter_context(tc.tile_pool(name="x", bufs=3))
        ypool = ctx.enter_context(tc.tile_pool(name="y", bufs=3))
        ppool = ctx.enter_context(tc.tile_pool(name="ps", bufs=8, space="PSUM"))

        wt = cpool.tile([P, 3, 2, 256], MM_DT)
        nc.gpsimd.dma_start(wt[:], wts[:])

        tile0 = 0
        for u in units:
            ucol = u * FT
            base = tile0 * FT
            xt = xpool.tile([P, 2, ucol + 2], MM_DT)
            nc.sync.dma_start(xt[:], xin[:, :, base : base + ucol + 2])
            yt = ypool.tile([P, 2, ucol], MM_DT)
            for f in range(u):
                for o in range(2):
                    ps = ppool.tile([P, FT], mybir.dt.float32)
                    g = 0
                    for t in range(3):
                        for k in range(2):
                            lhsT = wt[:, t, k, o * P : (o + 1) * P]
                            rhs = xt[:, k, f * FT + t : f * FT + t + FT]
                            nc.tensor.matmul(
                                ps[:], lhsT, rhs, start=(g == 0), stop=(g == 5)
                            )
                            g += 1
                    nc.vector.tensor_copy(yt[:, o, f * FT : (f + 1) * FT], ps[:])
            nc.sync.dma_start(yout[:, :, base : base + ucol], yt[:])
            tile0 += u
    _split_drain_waits(nc)
    return nc


def pack_inputs(xs, weight):
    """xs: list of 4 arrays [256, 256, 15, 15] fp32. in_maps for cores 0-7."""
    # w_dev[p, t, k, o] = weight[o, k*128+p, t]
    w_dev = np.ascontiguousarray(
        weight.transpose(1, 2, 0).reshape(2, P, 3, 256).transpose(1, 2, 0, 3)
    ).astype(MM_NP)

    C = 256
    xflat = np.zeros((C, NCORE * CORE_COLS), MM_NP)
    for d in range(4):
        xflat[:, _COLMAP[d].reshape(-1)] = (
            xs[d].transpose(1, 0, 2, 3).reshape(C, -1).astype(MM_NP)
        )

    in_maps = []
    for core in range(NCORE):
        seg = xflat[:, core * CORE_COLS : (core + 1) * CORE_COLS]
        xin_np = np.zeros((P, 2, CORE_COLS + 2), MM_NP)
        xin_np[:, 0, 1 : CORE_COLS + 1] = seg[:P]
        xin_np[:, 1, 1 : CORE_COLS + 1] = seg[P:]
        in_maps.append({"xin": xin_np, "wts": w_dev})
    return in_maps


def unpack_outputs(results, bias):
    O = 256
    yflat = np.empty((O, NCORE * CORE_COLS), np.float32)
    for core in range(NCORE):
        yo = np.asarray(results[core]["yout"])        # [128, 2, CORE_COLS] fp16
        yflat[:, core * CORE_COLS : (core + 1) * CORE_COLS] = (
            yo.transpose(1, 0, 2).reshape(O, CORE_COLS).astype(np.float32)
        )
    outs = []
    b = bias[None, :, None].astype(np.float32)
    for d in range(4):
        yd = yflat[:, _COLMAP[d].reshape(-1)].reshape(O, 256, H * W)
        yd = yd.transpose(1, 0, 2) + b
        outs.append(np.ascontiguousarray(yd.reshape(256, 256, H, W)))
    return tuple(outs)


def kernel(x0, x1, x2, x3, weight, bias):
    xs = [np.ascontiguousarray(np.asarray(a, dtype=np.float32)) for a in (x0, x1, x2, x3)]
    weight = np.asarray(weight, dtype=np.float32)
    bias = np.asarray(bias, dtype=np.float32)

    nc = build_program()
    in_maps = pack_inputs(xs, weight)
    res = run_bass_kernel_spmd(nc, in_maps, list(range(NCORE)))
    return unpack_outputs(res.results, bias)
